# revision 1
# baseline (speedup 1.0000x reference)
"""Trainium2 Bass kernel for nn_Attention_85658827752062 (sparse_attention).

Math (per batch b, head h):
    w[t]   = sum_d q[b,h,d] * past_k[b,h,d,t]      (t < 8192)
    w_new  = sum_d q[b,h,d] * k[b,h,d]
    res[d] = sum_t w[t] * past_v[b,h,t,d] + w_new * v[b,h,d]

Sharding: tensor-parallel over heads. 32 heads / 8 cores = 4 heads per core.
No cross-device communication; host slices inputs and concatenates outputs.

Per-core kernel design (memory-bound; roofline = stream 256 MiB of past_k/
past_v per core at ~358 GB/s ≈ 750 us/chip):
  - Heads are processed in pairs so the K-side matmul uses all 128 partitions.
  - K side: lhsT = K2 tile [128(=2 heads x 64 d), 128 t-cols] (stationary),
    rhs = q2 block-diagonal [128, 2] -> psum wT[tcol, head]. The t-columns of
    each matmul j are strided (t = 64*c + j) so that wT comes out in the
    permuted order that matches the V-side SBUF layout below.
  - V side: past_v[b,h] ([8192,64], t-major => contiguous 16 KiB per
    partition when partition p holds t in [64p, 64p+64)). For each j:
    lhsT = wT[:, j] [128,1] (stationary), rhs = V[128, 64] -> accumulate
    res[1, 64] in PSUM.
  - The fresh-token (k, v) contribution is two extra tiny matmuls.
  - All TensorE compute in bf16 (fp32 matmul is 4x slower); DVE casts the
    streamed f32 tiles to bf16 on-chip. Accumulation stays fp32 in PSUM.
"""

import os
import sys

import numpy as np

for _p in ("/opt/trn_rl_repo", "/root/.axon_site/_ro/trn_rl_repo"):
    if os.path.isdir(_p) and _p not in sys.path:
        sys.path.append(_p)

import ml_dtypes  # noqa: E402

B, NX, T, HD = 16, 2048, 8192, 64
H = NX // HD               # 32 heads
N_CORES = 8
HPC = H // N_CORES         # 4 heads per core
NPC = HPC * HD             # 256 nx-columns per core
NPAIR = HPC // 2           # 2 head-pairs per core
JT = 64                    # t_lo values (j) per partition block
CT = T // JT               # 128 t-columns per K-side matmul
TC = T // 2                # K dma chunk size (t columns per chunk)
VF = T * HD // 128         # 4096 free elems per partition for a V tile

LAST_EXEC_NS = None
_CACHE = {}


def _build_nc():
    from concourse import bacc, tile
    import concourse.mybir as mybir

    F32 = mybir.dt.float32
    BF16 = mybir.dt.bfloat16

    nc = bacc.Bacc(
        "TRN2", target_bir_lowering=False, debug=False, num_devices=N_CORES
    )
    pk = nc.dram_tensor("past_k", [B, HPC, HD, T], F32, kind="ExternalInput").ap()
    pv = nc.dram_tensor("past_v", [B, HPC, T, HD], F32, kind="ExternalInput").ap()
    q2 = nc.dram_tensor("q2", [128, B * HPC], BF16, kind="ExternalInput").ap()
    k2 = nc.dram_tensor("k2", [128, B * NPAIR], BF16, kind="ExternalInput").ap()
    vnew = nc.dram_tensor("vnew", [1, B * NPC], BF16, kind="ExternalInput").ap()
    out = nc.dram_tensor("out", [B, NPC], F32, kind="ExternalOutput").ap()

    with tile.TileContext(nc) as tc:
        with (
            tc.tile_pool(name="kraw_p", bufs=3) as kraw_p,
            tc.tile_pool(name="kb_p", bufs=2) as kb_p,
            tc.tile_pool(name="vraw_p", bufs=3) as vraw_p,
            tc.tile_pool(name="vb_p", bufs=2) as vb_p,
            tc.tile_pool(name="wt_p", bufs=2) as wt_p,
            tc.tile_pool(name="small_p", bufs=1) as small_p,
            tc.tile_pool(name="out_p", bufs=2) as out_p,
            tc.tile_pool(name="pswt_p", bufs=2, space="PSUM") as pswt_p,
            tc.tile_pool(name="psres_p", bufs=2, space="PSUM") as psres_p,
        ):
            q2s = small_p.tile([128, B * HPC], BF16)
            nc.sync.dma_start(out=q2s[:], in_=q2)
            k2s = small_p.tile([128, B * NPAIR], BF16)
            nc.sync.dma_start(out=k2s[:], in_=k2)
            vns = small_p.tile([1, B * NPC], BF16)
            nc.sync.dma_start(out=vns[:], in_=vnew)

            for b in range(B):
                out_sb = out_p.tile([1, NPC], F32)
                for p in range(NPAIR):
                    # ---- K side: w[t] for both heads of the pair ----
                    kb = kb_p.tile([128, T], BF16)
                    for ci in range(T // TC):
                        kraw = kraw_p.tile([128, TC], F32)
                        nc.sync.dma_start(
                            out=kraw[:],
                            in_=pk[
                                b, 2 * p : 2 * p + 2, :, ci * TC : (ci + 1) * TC
                            ].rearrange("h d t -> (h d) t"),
                        )
                        nc.vector.tensor_copy(
                            kb[:, ci * TC : (ci + 1) * TC], kraw[:]
                        )

                    ps_wt = pswt_p.tile([128, 2 * JT + 2], F32)
                    kbv = kb.rearrange("p (c j) -> p c j", j=JT)
                    qcols = q2s[:, (b * NPAIR + p) * 2 : (b * NPAIR + p) * 2 + 2]
                    for j in range(JT):
                        nc.tensor.matmul(
                            ps_wt[:, 2 * j : 2 * j + 2],
                            kbv[:, :, j],
                            qcols,
                            start=True,
                            stop=True,
                        )
                    # fresh-token scores w_new for both heads -> cols 128:130
                    nc.tensor.matmul(
                        ps_wt[0:1, 2 * JT : 2 * JT + 2],
                        k2s[:, b * NPAIR + p : b * NPAIR + p + 1],
                        qcols,
                        start=True,
                        stop=True,
                    )
                    wt = wt_p.tile([128, 2 * JT + 2], BF16)
                    nc.vector.tensor_copy(wt[:], ps_wt[:])

                    # ---- V side: res[d] per head ----
                    for h in range(2):
                        vraw = vraw_p.tile([128, VF], F32)
                        nc.sync.dma_start(
                            out=vraw[:],
                            in_=pv[b, 2 * p + h].rearrange(
                                "(pp r) d -> pp (r d)", pp=128
                            ),
                        )
                        vb = vb_p.tile([128, VF], BF16)
                        nc.vector.tensor_copy(vb[:], vraw[:])

                        ps_res = psres_p.tile([1, HD], F32)
                        for j in range(JT):
                            nc.tensor.matmul(
                                ps_res[:],
                                wt[:, 2 * j + h : 2 * j + h + 1],
                                vb[:, j * HD : (j + 1) * HD],
                                start=(j == 0),
                                stop=False,
                            )
                        voff = (b * HPC + 2 * p + h) * HD
                        nc.tensor.matmul(
                            ps_res[:],
                            wt[0:1, 2 * JT + h : 2 * JT + h + 1],
                            vns[0:1, voff : voff + HD],
                            start=False,
                            stop=True,
                        )
                        nc.vector.tensor_copy(
                            out_sb[0:1, (2 * p + h) * HD : (2 * p + h + 1) * HD],
                            ps_res[:],
                        )
                nc.sync.dma_start(out=out[b : b + 1, :], in_=out_sb[:])

    nc.compile()
    return nc


def _get_nc():
    if "nc" not in _CACHE:
        _CACHE["nc"] = _build_nc()
    return _CACHE["nc"]


def _pack_core_inputs(c, q, k, v, past_k, past_v):
    bf16 = ml_dtypes.bfloat16
    h0 = c * HPC
    # q2[col*64+d, b*HPC + p*2 + col] = q[b, (h0 + 2p + col)*64 + d]
    qc = q[:, h0 * HD : (h0 + HPC) * HD].reshape(B, HPC, HD)  # [b, lh, d]
    q2 = np.zeros((128, B, NPAIR, 2), dtype=np.float32)
    for col in range(2):
        # heads with lh % 2 == col -> [b, p, d] -> [d, b, p]
        q2[col * 64 : (col + 1) * 64, :, :, col] = qc[:, col::2, :].transpose(
            2, 0, 1
        )
    q2 = q2.reshape(128, B * HPC).astype(bf16)

    # k2[part, b*NPAIR+p] = k[b, h0*HD + p*128 + part]
    kc = k[:, h0 * HD : (h0 + HPC) * HD].reshape(B, NPAIR, 128)
    k2 = np.ascontiguousarray(kc.transpose(2, 0, 1).reshape(128, B * NPAIR)).astype(
        bf16
    )

    vn = np.ascontiguousarray(v[:, h0 * HD : (h0 + HPC) * HD]).reshape(
        1, B * NPC
    ).astype(bf16)

    pk = np.ascontiguousarray(past_k[:, h0 : h0 + HPC])
    pv = np.ascontiguousarray(past_v[:, h0 : h0 + HPC])
    return {"past_k": pk, "past_v": pv, "q2": q2, "k2": k2, "vnew": vn}


def kernel(q, k, v, past_k, past_v):
    global LAST_EXEC_NS
    from concourse import bass_utils

    q = np.asarray(q, dtype=np.float32)
    k = np.asarray(k, dtype=np.float32)
    v = np.asarray(v, dtype=np.float32)
    past_k = np.asarray(past_k, dtype=np.float32)
    past_v = np.asarray(past_v, dtype=np.float32)

    nc = _get_nc()
    in_maps = [
        _pack_core_inputs(c, q, k, v, past_k, past_v) for c in range(N_CORES)
    ]

    trace = bool(int(os.environ.get("BASS_KERNEL_TRACE", "0")))
    if trace:
        # shim the NTFF profile hook (image's antenv lacks axon_hooks)
        import types
        import antenv

        if "antenv.axon_hooks" not in sys.modules:
            from trn_agent_boot.trn_boot import _ntff_profile_via_ctypes

            mod = types.ModuleType("antenv.axon_hooks")
            hook = _ntff_profile_via_ctypes("/opt/axon/libaxon_pjrt.so")
            mod.get_axon_ntff_profile_hook = lambda: hook
            sys.modules["antenv.axon_hooks"] = mod
            setattr(antenv, "axon_hooks", mod)
        bass_utils.upload_artifacts = lambda tmpdir: f"local://{tmpdir}"

    res = bass_utils.run_bass_kernel_spmd(
        nc, in_maps, core_ids=list(range(N_CORES)), trace=trace
    )
    LAST_EXEC_NS = res.exec_time_ns

    out = np.empty((B, NX), dtype=np.float32)
    for c in range(N_CORES):
        out[:, c * NPC : (c + 1) * NPC] = res.results[c]["out"]
    return out


# revision 2
# speedup vs baseline: 1.2281x; 1.2281x over previous
"""Trainium2 Bass kernel for nn_Attention_85658827752062 (sparse_attention).

Math (per batch b, head h):
    w[t]   = sum_d q[b,h,d] * past_k[b,h,d,t]      (t < 8192)
    w_new  = sum_d q[b,h,d] * k[b,h,d]
    res[d] = sum_t w[t] * past_v[b,h,t,d] + w_new * v[b,h,d]

Sharding: tensor-parallel over heads. 32 heads / 8 cores = 4 heads per core.
No cross-device communication; host slices inputs and concatenates outputs.

Per-core kernel design (memory-bound; roofline = stream 256 MiB of past_k/
past_v per core at ~358 GB/s ≈ 750 us/chip):
  - Heads are processed in pairs so the K-side matmul uses all 128 partitions.
  - K side: lhsT = K2 tile [128(=2 heads x 64 d), 128 t-cols] (stationary),
    rhs = q2 block-diagonal [128, 2] -> psum wT[tcol, head]. The t-columns of
    each matmul j are strided (t = 64*c + j) so that wT comes out in the
    permuted order that matches the V-side SBUF layout below.
  - V side: past_v[b,h] ([8192,64], t-major => contiguous 16 KiB per
    partition when partition p holds t in [64p, 64p+64)). For each j:
    lhsT = wT[:, j] [128,1] (stationary), rhs = V[128, 64] -> accumulate
    res[1, 64] in PSUM.
  - The fresh-token (k, v) contribution is two extra tiny matmuls.
  - All TensorE compute in bf16 (fp32 matmul is 4x slower). The f32->bf16
    conversion happens inside the load DMA (SWDGE cast) so no DVE pass over
    the bulk data is needed. Accumulation stays fp32 in PSUM.
  - Output DMAs go on the scalar-engine HWDGE ring so they never block the
    input-prefetch stream (the sync ring is FIFO per engine).
"""

import os
import sys

import numpy as np

for _p in ("/opt/trn_rl_repo", "/root/.axon_site/_ro/trn_rl_repo"):
    if os.path.isdir(_p) and _p not in sys.path:
        sys.path.append(_p)

import ml_dtypes  # noqa: E402

B, NX, T, HD = 16, 2048, 8192, 64
H = NX // HD               # 32 heads
N_CORES = 8
HPC = H // N_CORES         # 4 heads per core
NPC = HPC * HD             # 256 nx-columns per core
NPAIR = HPC // 2           # 2 head-pairs per core
JT = 64                    # t_lo values (j) per partition block
CT = T // JT               # 128 t-columns per K-side matmul
TC = T // 2                # K dma chunk size (t columns per chunk)
VF = T * HD // 128         # 4096 free elems per partition for a V tile

USE_DMA_CAST = bool(int(os.environ.get("BASS_KERNEL_DMA_CAST", "1")))

LAST_EXEC_NS = None
_CACHE = {}


def _build_nc():
    from concourse import bacc, tile
    import concourse.mybir as mybir

    F32 = mybir.dt.float32
    BF16 = mybir.dt.bfloat16

    nc = bacc.Bacc(
        "TRN2", target_bir_lowering=False, debug=False, num_devices=N_CORES
    )
    pk = nc.dram_tensor("past_k", [B, HPC, HD, T], F32, kind="ExternalInput").ap()
    pv = nc.dram_tensor("past_v", [B, HPC, T, HD], F32, kind="ExternalInput").ap()
    q2 = nc.dram_tensor("q2", [128, B * HPC], BF16, kind="ExternalInput").ap()
    k2 = nc.dram_tensor("k2", [128, B * NPAIR], BF16, kind="ExternalInput").ap()
    vnew = nc.dram_tensor("vnew", [1, B * NPC], BF16, kind="ExternalInput").ap()
    out = nc.dram_tensor("out", [B, NPC], F32, kind="ExternalOutput").ap()

    with tile.TileContext(nc) as tc:
        with (
            tc.tile_pool(name="kraw_p", bufs=3) as kraw_p,
            tc.tile_pool(name="kb_p", bufs=3) as kb_p,
            tc.tile_pool(name="vraw_p", bufs=3) as vraw_p,
            tc.tile_pool(name="vb_p", bufs=4) as vb_p,
            tc.tile_pool(name="wt_p", bufs=2) as wt_p,
            tc.tile_pool(name="small_p", bufs=1) as small_p,
            tc.tile_pool(name="out_p", bufs=2) as out_p,
            tc.tile_pool(name="pswt_p", bufs=2, space="PSUM") as pswt_p,
            tc.tile_pool(name="psres_p", bufs=2, space="PSUM") as psres_p,
        ):
            q2s = small_p.tile([128, B * HPC], BF16)
            nc.scalar.dma_start(out=q2s[:], in_=q2)
            k2s = small_p.tile([128, B * NPAIR], BF16)
            nc.scalar.dma_start(out=k2s[:], in_=k2)
            vns = small_p.tile([1, B * NPC], BF16)
            nc.scalar.dma_start(out=vns[:], in_=vnew)

            for b in range(B):
                out_sb = out_p.tile([1, NPC], F32)
                for p in range(NPAIR):
                    # ---- loads (and f32->bf16) ----
                    kb = kb_p.tile([128, T], BF16)
                    if USE_DMA_CAST:
                        for ci in range(T // TC):
                            nc.gpsimd.dma_start(
                                out=kb[:, ci * TC : (ci + 1) * TC],
                                in_=pk[
                                    b, 2 * p : 2 * p + 2, :, ci * TC : (ci + 1) * TC
                                ].rearrange("h d t -> (h d) t"),
                            )
                        vbs = []
                        for h in range(2):
                            vb = vb_p.tile([128, VF], BF16, name=f"vb{h}")
                            nc.gpsimd.dma_start(
                                out=vb[:],
                                in_=pv[b, 2 * p + h].rearrange(
                                    "(pp r) d -> pp (r d)", pp=128
                                ),
                            )
                            vbs.append(vb)
                    else:
                        for ci in range(T // TC):
                            kraw = kraw_p.tile([128, TC], F32)
                            nc.sync.dma_start(
                                out=kraw[:],
                                in_=pk[
                                    b, 2 * p : 2 * p + 2, :, ci * TC : (ci + 1) * TC
                                ].rearrange("h d t -> (h d) t"),
                            )
                            nc.vector.tensor_copy(
                                kb[:, ci * TC : (ci + 1) * TC], kraw[:]
                            )
                        vbs = []
                        for h in range(2):
                            vraw = vraw_p.tile([128, VF], F32, name=f"vraw{h}")
                            nc.sync.dma_start(
                                out=vraw[:],
                                in_=pv[b, 2 * p + h].rearrange(
                                    "(pp r) d -> pp (r d)", pp=128
                                ),
                            )
                            vb = vb_p.tile([128, VF], BF16, name=f"vb{h}")
                            nc.vector.tensor_copy(vb[:], vraw[:])
                            vbs.append(vb)

                    # ---- K side: w[t] for both heads of the pair ----
                    ps_wt = pswt_p.tile([128, 2 * JT + 2], F32)
                    kbv = kb.rearrange("p (c j) -> p c j", j=JT)
                    qcols = q2s[:, (b * NPAIR + p) * 2 : (b * NPAIR + p) * 2 + 2]
                    for j in range(JT):
                        nc.tensor.matmul(
                            ps_wt[:, 2 * j : 2 * j + 2],
                            kbv[:, :, j],
                            qcols,
                            start=True,
                            stop=True,
                        )
                    # fresh-token scores w_new for both heads -> cols 128:130
                    nc.tensor.matmul(
                        ps_wt[0:1, 2 * JT : 2 * JT + 2],
                        k2s[:, b * NPAIR + p : b * NPAIR + p + 1],
                        qcols,
                        start=True,
                        stop=True,
                    )
                    wt = wt_p.tile([128, 2 * JT + 2], BF16)
                    nc.vector.tensor_copy(wt[:], ps_wt[:])

                    # ---- V side: res[d] per head ----
                    ps_res_l = []
                    for h in range(2):
                        vb = vbs[h]
                        ps_res = psres_p.tile([1, HD], F32, name=f"ps_res{h}")
                        for j in range(JT):
                            nc.tensor.matmul(
                                ps_res[:],
                                wt[:, 2 * j + h : 2 * j + h + 1],
                                vb[:, j * HD : (j + 1) * HD],
                                start=(j == 0),
                                stop=False,
                            )
                        voff = (b * HPC + 2 * p + h) * HD
                        nc.tensor.matmul(
                            ps_res[:],
                            wt[0:1, 2 * JT + h : 2 * JT + h + 1],
                            vns[0:1, voff : voff + HD],
                            start=False,
                            stop=True,
                        )
                        ps_res_l.append(ps_res)
                    for h in range(2):
                        nc.vector.tensor_copy(
                            out_sb[0:1, (2 * p + h) * HD : (2 * p + h + 1) * HD],
                            ps_res_l[h],
                        )
                nc.scalar.dma_start(out=out[b : b + 1, :], in_=out_sb[:])

    nc.compile()
    return nc


def _get_nc():
    if "nc" not in _CACHE:
        _CACHE["nc"] = _build_nc()
    return _CACHE["nc"]


def _pack_core_inputs(c, q, k, v, past_k, past_v):
    bf16 = ml_dtypes.bfloat16
    h0 = c * HPC
    # q2[col*64+d, b*HPC + p*2 + col] = q[b, (h0 + 2p + col)*64 + d]
    qc = q[:, h0 * HD : (h0 + HPC) * HD].reshape(B, HPC, HD)  # [b, lh, d]
    q2 = np.zeros((128, B, NPAIR, 2), dtype=np.float32)
    for col in range(2):
        # heads with lh % 2 == col -> [b, p, d] -> [d, b, p]
        q2[col * 64 : (col + 1) * 64, :, :, col] = qc[:, col::2, :].transpose(
            2, 0, 1
        )
    q2 = q2.reshape(128, B * HPC).astype(bf16)

    # k2[part, b*NPAIR+p] = k[b, h0*HD + p*128 + part]
    kc = k[:, h0 * HD : (h0 + HPC) * HD].reshape(B, NPAIR, 128)
    k2 = np.ascontiguousarray(kc.transpose(2, 0, 1).reshape(128, B * NPAIR)).astype(
        bf16
    )

    vn = np.ascontiguousarray(v[:, h0 * HD : (h0 + HPC) * HD]).reshape(
        1, B * NPC
    ).astype(bf16)

    pk = np.ascontiguousarray(past_k[:, h0 : h0 + HPC])
    pv = np.ascontiguousarray(past_v[:, h0 : h0 + HPC])
    return {"past_k": pk, "past_v": pv, "q2": q2, "k2": k2, "vnew": vn}


def kernel(q, k, v, past_k, past_v):
    global LAST_EXEC_NS
    from concourse import bass_utils

    q = np.asarray(q, dtype=np.float32)
    k = np.asarray(k, dtype=np.float32)
    v = np.asarray(v, dtype=np.float32)
    past_k = np.asarray(past_k, dtype=np.float32)
    past_v = np.asarray(past_v, dtype=np.float32)

    nc = _get_nc()
    in_maps = [
        _pack_core_inputs(c, q, k, v, past_k, past_v) for c in range(N_CORES)
    ]

    trace = bool(int(os.environ.get("BASS_KERNEL_TRACE", "0")))
    if trace:
        # shim the NTFF profile hook (image's antenv lacks axon_hooks)
        import types
        import antenv

        if "antenv.axon_hooks" not in sys.modules:
            from trn_agent_boot.trn_boot import _ntff_profile_via_ctypes

            mod = types.ModuleType("antenv.axon_hooks")
            hook = _ntff_profile_via_ctypes("/opt/axon/libaxon_pjrt.so")
            mod.get_axon_ntff_profile_hook = lambda: hook
            sys.modules["antenv.axon_hooks"] = mod
            setattr(antenv, "axon_hooks", mod)
        bass_utils.upload_artifacts = lambda tmpdir: f"local://{tmpdir}"

    res = bass_utils.run_bass_kernel_spmd(
        nc, in_maps, core_ids=list(range(N_CORES)), trace=trace
    )
    LAST_EXEC_NS = res.exec_time_ns

    out = np.empty((B, NX), dtype=np.float32)
    for c in range(N_CORES):
        out[:, c * NPC : (c + 1) * NPC] = res.results[c]["out"]
    return out
